# revision 37
# baseline (speedup 1.0000x reference)
"""Multi-head self-attention (B=4, T=2048, D=1024, H=16, Dh=64) on 8 trn2 cores.

Sharding: core c = (batch b = c//2, head-half = c%2). Each core computes the
attention output contribution of 8 heads for one batch element, including the
row-sharded output projection; the host sums the two half-partials per batch
and adds the folded bias (bv @ Wo + bo).

Per-core dataflow (fp16 matmuls, fp32 PSUM accumulation, fp8 AV):
  xT [D,T] -> QT/KT [512,T] fp16 (scale 1/8 folded into Wq), V fp8e4 [T,512]
  per head pair (A,B), per 512-wide q chunk, per k token-tile t:
    scoresT[:,A/B] = KT-slice^T x QT-slice   (K=64, A/B concurrent on
    disjoint PE row halves)
    E = exp(scoresT) on ScalarE, PSUM f32 -> SBUF fp8e4, written into the
    t%2 slot of a per-token-pair E tile
  per token-tile pair tp: DoubleRow fp8 matmuls contract both tiles at once:
    num_A/B += V_aug[tp]^T @ E[tp]  (V carries a ones column -> rowsum lands
    in num row 64 (A) / 32 (B)); 2x fewer AV matmul cycles than fp16.
  softmax denominator: copy nums PSUM->SBUF f32 (releases banks), reciprocal
  of the rowsum rows into a [2,QC] fp16 tile, broadcast to 128 partitions
  with a single K=2 fp16 matmul (mask lhsT), multiply -> AOT [512,T] fp16;
  out = AOT^T-slices @ Wo -> PSUM f32 -> out DRAM f32.

Emission is software-pipelined as in the bf16 baseline: projections and the
output projection of finished q-chunks are sprinkled into the per-k-tile
slots of the attention loop; the normalize chain of chunk i is emitted at the
top of chunk i+1.
"""

import numpy as np
import ml_dtypes

P = 128
T = 2048
D = 1024
DH = 512          # per-core head dims (8 heads x 64)
NK = D // P       # 8 contraction tiles for projections
NT = T // P       # 16 token tiles
NTP = NT // 2     # 8 token-tile pairs (fp8 DoubleRow AV)
QC = 512          # q-chunk width
NQC = T // QC     # 4
NPAIR = 4         # head pairs per core
F16 = np.float16
F8 = ml_dtypes.float8_e4m3

_CACHED_NC = None


def build_nc():
    global _CACHED_NC
    if _CACHED_NC is not None:
        return _CACHED_NC
    from contextlib import ExitStack
    import concourse.mybir as mybir
    import concourse.tile as tile
    from concourse import bacc
    from concourse.bass import ds

    f32 = mybir.dt.float32
    fp16 = mybir.dt.float16
    fp8 = mybir.dt.float8e4
    EXP = mybir.ActivationFunctionType.Exp
    DR = mybir.MatmulPerfMode.DoubleRow

    nc = bacc.Bacc("TRN2", target_bir_lowering=False, debug=False, num_devices=8)
    xt_d = nc.dram_tensor("xt", [D, T], fp16, kind="ExternalInput")
    wq_d = nc.dram_tensor("wq", [D, DH], fp16, kind="ExternalInput")
    wk_d = nc.dram_tensor("wk", [D, DH], fp16, kind="ExternalInput")
    wv_d = nc.dram_tensor("wv", [D, DH], fp16, kind="ExternalInput")
    wo_d = nc.dram_tensor("wo", [DH, D], fp16, kind="ExternalInput")
    bq_d = nc.dram_tensor("bq2", [P, NPAIR], f32, kind="ExternalInput")
    bk_d = nc.dram_tensor("bk2", [P, NPAIR], f32, kind="ExternalInput")
    out_d = nc.dram_tensor("out", [T, D], f32, kind="ExternalOutput")

    with tile.TileContext(nc) as tc, ExitStack() as ctx:
        cpool = ctx.enter_context(tc.tile_pool(name="const", bufs=1))
        pp = ctx.enter_context(tc.tile_pool(name="proj", bufs=2, space="PSUM"))
        ps_s = ctx.enter_context(tc.tile_pool(name="scores", bufs=2, space="PSUM"))
        ps_n = ctx.enter_context(tc.tile_pool(name="num", bufs=1, space="PSUM"))
        epool = ctx.enter_context(tc.tile_pool(name="esb", bufs=3))
        npool = ctx.enter_context(tc.tile_pool(name="nsb", bufs=2))
        rpool = ctx.enter_context(tc.tile_pool(name="rsb", bufs=2))
        opool = ctx.enter_context(tc.tile_pool(name="osb", bufs=3))

        xt = [cpool.tile([P, T], fp16, name=f"xt{k}", tag=f"xt{k}") for k in range(NK)]
        wq = cpool.tile([P, NK, DH], fp16, name="wq_s", tag="wq_s")
        wk = cpool.tile([P, NK, DH], fp16, name="wk_s", tag="wk_s")
        wv = cpool.tile([P, NK, DH], fp16, name="wv_s", tag="wv_s")
        wo = cpool.tile([P, NPAIR, D], fp16, name="wo_s", tag="wo_s")
        bq = cpool.tile([P, NPAIR], f32, name="bq_s", tag="bq_s")
        bk = cpool.tile([P, NPAIR], f32, name="bk_s", tag="bk_s")
        # rank-1 lhsT for the denominator broadcast matmuls
        ones1 = cpool.tile([1, 64], fp16, name="ones1", tag="ones1")
        qt = [cpool.tile([P, T], fp16, name=f"qt{r}", tag=f"qt{r}") for r in range(NPAIR)]
        kt = [cpool.tile([P, T], fp16, name=f"kt{r}", tag=f"kt{r}") for r in range(NPAIR)]
        # V layout (fp8e4), two contiguous [.., 2, 128] lhsT blocks per
        # (token-tile-pair tp, head pair r) for DoubleRow weight loads:
        #   A block: [v_A(64), ones, zeros(63)] -> num rows 0:63 = A,
        #            row 64 = rowsum_A
        #   B block: [zeros(32), ones, zeros(31), v_B(64)] -> num rows
        #            64:127 = B, row 32 = rowsum_B
        vsa = cpool.tile([P, NTP, NPAIR, 2, P], fp8, name="va_s", tag="va_s")
        vsbb = cpool.tile([P, NTP, NPAIR, 2, P], fp8, name="vb_s", tag="vb_s")
        aot = [cpool.tile([P, T], fp16, name=f"aot{r}", tag=f"aot{r}") for r in range(NPAIR)]

        # loads: xt early (first projection chains need it), wv/wo last
        nc.sync.dma_start(wq[:], wq_d[:].rearrange("(k p) n -> p k n", p=P))
        nc.sync.dma_start(bq[:], bq_d[:])
        for k in range(NK):
            nc.sync.dma_start(xt[k][:], xt_d[ds(k * P, P), :])
        nc.sync.dma_start(wk[:], wk_d[:].rearrange("(k p) n -> p k n", p=P))
        nc.sync.dma_start(bk[:], bk_d[:])
        nc.sync.dma_start(wv[:], wv_d[:].rearrange("(k p) n -> p k n", p=P))
        nc.sync.dma_start(wo[:], wo_d[:].rearrange("(r p) n -> p r n", p=P))
        nc.vector.memset(ones1[:], 1.0)
        nc.gpsimd.memset(vsa[:, :, :, :, 64:65], 1.0)
        nc.gpsimd.memset(vsa[:, :, :, :, 65:P], 0.0)
        nc.gpsimd.memset(vsbb[:, :, :, :, 0:32], 0.0)
        nc.gpsimd.memset(vsbb[:, :, :, :, 32:33], 1.0)
        nc.gpsimd.memset(vsbb[:, :, :, :, 33:64], 0.0)

        # projection chains are emitted in two halves so a sprinkled chain
        # never inserts more than ~1us of PE work between attention slots
        def proj_qk_a(dst, w, b, r, qc):
            ps = pp.tile([P, QC], f32, name="ps_p", tag="ps_p")
            for k in range(NK // 2):
                nc.tensor.matmul(ps[:], w[:, k, ds(r * P, P)], xt[k][:, ds(qc * QC, QC)],
                                 start=(k == 0), stop=False)
            return ps

        def proj_qk_b(ps, dst, w, b, r, qc):
            for k in range(NK // 2, NK):
                nc.tensor.matmul(ps[:], w[:, k, ds(r * P, P)], xt[k][:, ds(qc * QC, QC)],
                                 start=False, stop=(k == NK - 1))
            nc.vector.tensor_scalar_add(dst[:, ds(qc * QC, QC)], ps[:], b[:, r:r + 1])

        def proj_qk(dst, w, b, r, qc):
            proj_qk_b(proj_qk_a(dst, w, b, r, qc), dst, w, b, r, qc)

        def proj_v_a(t):
            ps = pp.tile([P, DH], f32, name="ps_p", tag="ps_p")
            for k in range(NK // 2):
                nc.tensor.matmul(ps[:], xt[k][:, ds(t * P, P)], wv[:, k, :],
                                 start=(k == 0), stop=False)
            return ps

        def proj_v_b(ps, t):
            for k in range(NK // 2, NK):
                nc.tensor.matmul(ps[:], xt[k][:, ds(t * P, P)], wv[:, k, :],
                                 start=False, stop=(k == NK - 1))
            psv = ps.rearrange("p (r hd) -> p r hd", r=NPAIR)
            nc.vector.tensor_copy(vsa[:, t // 2, :, t % 2, 0:64], psv[:, :, 0:64])
            nc.vector.tensor_copy(vsbb[:, t // 2, :, t % 2, 64:P], psv[:, :, 64:128])

        def proj_v(t):
            proj_v_b(proj_v_a(t), t)

        def proj_out(j, n):
            ps = pp.tile([P, QC], f32, name="ps_p", tag="ps_p")
            for r in range(NPAIR):
                nc.tensor.matmul(ps[:], aot[r][:, ds(j * P, P)], wo[:, r, ds(n * QC, QC)],
                                 start=(r == 0), stop=(r == NPAIR - 1))
            o = opool.tile([P, QC], f32, name="ost", tag="ost")
            nc.vector.tensor_copy(o[:], ps[:])
            nc.sync.dma_start(out_d[ds(j * P, P), ds(n * QC, QC)], o[:])

        def emit_norm_copies(st):
            nA, nB, r, qc = st
            # PSUM -> SBUF copies release the num banks promptly; the rest of
            # the chain runs off the critical path.
            # on the Scalar engine: it idles at window boundaries while the
            # DVE queue is backed up with drained projection copies, and the
            # next window's first AV is blocked on these bank releases
            cA = npool.tile([65, QC], f32, name="cA", tag="cA")
            cB = npool.tile([P, QC], f32, name="cB", tag="cB")
            nc.vector.tensor_copy(cA[:], nA[0:65, :])
            nc.vector.tensor_copy(cB[:], nB[:])
            return (cA, cB, r, qc)

        def emit_norm_rest(st):
            cA, cB, r, qc = st
            r2a = rpool.tile([1, QC], f32, name="r2a", tag="r2a")
            r2b = rpool.tile([1, QC], f32, name="r2b", tag="r2b")
            nc.vector.tensor_copy(r2a[:], cA[64:65, :])
            nc.vector.tensor_copy(r2b[:], cB[32:33, :])
            r3af = rpool.tile([1, QC], f32, name="r3af", tag="r3af")
            r3bf = rpool.tile([1, QC], f32, name="r3bf", tag="r3bf")
            nc.vector.reciprocal_approx_fast(r3af[:], r2a[:])
            nc.vector.reciprocal_approx_fast(r3bf[:], r2b[:])
            r3a = rpool.tile([1, QC], fp16, name="r3a", tag="r3a")
            r3b = rpool.tile([1, QC], fp16, name="r3b", tag="r3b")
            nc.vector.tensor_copy(r3a[:], r3af[:])
            nc.vector.tensor_copy(r3b[:], r3bf[:])
            rbc = pp.tile([P, QC], f32, name="rbc", tag="ps_p")
            nc.tensor.matmul(rbc[0:64, :], ones1[:], r3a[:], start=True, stop=True)
            nc.tensor.matmul(rbc[64:P, :], ones1[:], r3b[:], start=True, stop=True)
            nc.vector.tensor_mul(aot[r][0:64, ds(qc * QC, QC)], cA[0:64, :], rbc[0:64, :])
            nc.vector.tensor_mul(aot[r][64:P, ds(qc * QC, QC)], cB[64:P, :], rbc[64:P, :])

        def emit_normalize(st):
            emit_norm_rest(emit_norm_copies(st))

        def qk_units(dst, w, b, r, qc):
            cell = []
            return [lambda: cell.append(proj_qk_a(dst, w, b, r, qc)),
                    lambda: proj_qk_b(cell[0], dst, w, b, r, qc)]

        def v_units(t):
            cell = []
            return [lambda t=t: cell.append(proj_v_a(t)),
                    lambda t=t: proj_v_b(cell[0], t)]

        # startup: KT spans all k tokens, so the first KT-pair0 chain (plus
        # the first QT chunk and the first two V token-tiles) precede
        # attention; everything else is sprinkled into attention slots.
        proj_qk(qt[0], wq, bq, 0, 0)
        proj_qk(kt[0], wk, bk, 0, 0)
        proj_v(0)
        proj_v(1)

        pending_norm = None
        for r in range(NPAIR):
            for qc in range(NQC):
                # unit queue for this (r, qc) window: each unit is ~1us of
                # PE work, drained a few per slot
                units = []
                if r == 0 and qc == 0:
                    units += qk_units(kt[0], wk, bk, 0, 1)
                    units += v_units(2) + v_units(3)
                    units += qk_units(kt[0], wk, bk, 0, 2)
                    units += v_units(4) + v_units(5)
                    units += qk_units(kt[0], wk, bk, 0, 3)
                    for t in range(6, 9):
                        units += v_units(t)
                    units += qk_units(qt[0], wq, bq, 0, 1)
                    for t in range(9, 12):
                        units += v_units(t)
                    units += qk_units(qt[0], wq, bq, 0, 2)
                    for t in range(12, NT):
                        units += v_units(t)
                    units += qk_units(qt[0], wq, bq, 0, 3)
                elif r == 0:
                    # pair-1 projections squeezed into the 3 remaining
                    # pair-0 windows (KT chains first)
                    chains = [("k", 0), ("k", 1), ("k", 2), ("k", 3),
                              ("q", 0), ("q", 1), ("q", 2), ("q", 3)]
                    for kind, c in chains[(qc - 1) * 3:qc * 3]:
                        dst, w_, b_ = ((kt[1], wk, bk) if kind == "k"
                                       else (qt[1], wq, bq))
                        units += qk_units(dst, w_, b_, 1, c)
                elif r < NPAIR - 1:
                    # next pair's projections: KT chains first (needed from
                    # t=0 of its qc0), QT chains later
                    kind = ("k", kt) if qc < 2 else ("q", qt)
                    w_, b_ = (wk, bk) if qc < 2 else (wq, bq)
                    c0 = 2 * qc if qc < 2 else 2 * (qc - 2)
                    units += qk_units(kind[1][r + 1], w_, b_, r + 1, c0)
                    units += qk_units(kind[1][r + 1], w_, b_, r + 1, c0 + 1)
                if r == NPAIR - 1 and qc > 0:
                    for i in range(8):
                        j = (qc - 1) * NPAIR + i // 2
                        units.append(lambda j=j, n=i % 2: proj_out(j, n))

                nA = nB = None
                es = {}
                ui = 0
                av_done = 0
                for t in range(NT):
                    sc = ps_s.tile([P, 2, QC], f32, name="sc", tag="sc")
                    nc.tensor.matmul(sc[:, 0, :], kt[r][0:64, ds(t * P, P)],
                                     qt[r][0:64, ds(qc * QC, QC)], start=True, stop=True)
                    nc.tensor.matmul(sc[:, 1, :], kt[r][64:P, ds(t * P, P)],
                                     qt[r][64:P, ds(qc * QC, QC)], start=True, stop=True)
                    if t % 2 == 0:
                        es[t // 2] = epool.tile([P, 2, 2, QC], fp8, name="eT", tag="eT")
                    # E slot layout: [partitions, head, t-in-pair, QC] so each
                    # head's two t-slots are a contiguous DoubleRow ifmap
                    nc.scalar.activation(es[t // 2][:, :, t % 2, :], sc[:], EXP)
                    if t == 0 and pending_norm is not None:
                        emit_normalize(pending_norm)
                        pending_norm = None
                    # AV pairs start at slot 4 (not 2) so the previous
                    # window's num-bank drain never blocks the first one
                    if t >= 4 and t % 2 == 0:
                        if t == 4:
                            nA = ps_n.tile([P, QC], f32, name="nA", tag="nA")
                            nB = ps_n.tile([P, QC], f32, name="nB", tag="nB")
                        while av_done <= t // 2 - 1:
                            tp = av_done
                            nc.tensor.matmul(nA[:], vsa[:, tp, r, :, :],
                                             es[tp][:, 0, :, :],
                                             start=(tp == 0), stop=False,
                                             perf_mode=DR)
                            nc.tensor.matmul(nB[:], vsbb[:, tp, r, :, :],
                                             es[tp][:, 1, :, :],
                                             start=(tp == 0), stop=False,
                                             perf_mode=DR)
                            del es[tp]
                            av_done += 1
                    # drain the unit queue evenly, finishing by slot 13 so
                    # the DVE queue is clear for the boundary num drain
                    want = -(-(len(units) - ui) // max(1, NT - 2 - t))
                    for _ in range(want):
                        units[ui]()
                        ui += 1
                while ui < len(units):
                    units[ui]()
                    ui += 1
                tp = NTP - 1
                nc.tensor.matmul(nA[:], vsa[:, tp, r, :, :], es[tp][:, 0, :, :],
                                 start=False, stop=True, perf_mode=DR)
                nc.tensor.matmul(nB[:], vsbb[:, tp, r, :, :], es[tp][:, 1, :, :],
                                 start=False, stop=True, perf_mode=DR)
                del es[tp]
                pending_norm = (nA, nB, r, qc)

        emit_normalize(pending_norm)
        # tail: output projection of the last q-chunk
        for j in range((NQC - 1) * NPAIR, NT):
            for n in range(D // QC):
                proj_out(j, n)

    nc.compile()

    _CACHED_NC = nc
    return nc


def prepare_in_maps(inputs):
    x = np.asarray(inputs["x"], np.float32)
    Wq = np.asarray(inputs["Wq"], np.float32)
    bq = np.asarray(inputs["bq"], np.float32)
    Wk = np.asarray(inputs["Wk"], np.float32)
    bk = np.asarray(inputs["bk"], np.float32)
    Wv = np.asarray(inputs["Wv"], np.float32)
    Wo = np.asarray(inputs["Wo"], np.float32)
    in_maps = []
    for c in range(8):
        b, half = c // 2, c % 2
        cols = slice(half * DH, (half + 1) * DH)
        in_maps.append({
            "xt": np.ascontiguousarray(x[b].T).astype(F16),
            "wq": np.ascontiguousarray(Wq[:, cols] / 8.0).astype(F16),
            "wk": np.ascontiguousarray(Wk[:, cols]).astype(F16),
            "wv": np.ascontiguousarray(Wv[:, cols]).astype(F16),
            "wo": np.ascontiguousarray(Wo[cols, :]).astype(F16),
            "bq2": np.ascontiguousarray((bq[cols] / 8.0).astype(np.float32).reshape(NPAIR, P).T),
            "bk2": np.ascontiguousarray(bk[cols].astype(np.float32).reshape(NPAIR, P).T),
        })
    return in_maps


def postprocess(results, inputs):
    bv = np.asarray(inputs["bv"], np.float64)
    Wo = np.asarray(inputs["Wo"], np.float64)
    bo = np.asarray(inputs["bo"], np.float64)
    bo_eff = (bv @ Wo + bo).astype(np.float32)
    out = np.empty((4, T, D), np.float32)
    for b in range(4):
        out[b] = (results[2 * b]["out"]
                  + results[2 * b + 1]["out"]
                  + bo_eff[None, :])
    return out


def kernel(**inputs):
    from concourse.bass_utils import run_bass_kernel_spmd
    nc = build_nc()
    in_maps = prepare_in_maps(inputs)
    res = run_bass_kernel_spmd(nc, in_maps, core_ids=list(range(8)))
    return postprocess(res.results, inputs)


# revision 41
# speedup vs baseline: 1.1530x; 1.1530x over previous
"""Multi-head self-attention (B=4, T=2048, D=1024, H=16, Dh=64) on 8 trn2 cores.

Sharding: core c = (batch b = c//2, head-half = c%2). Each core computes the
attention output contribution of 8 heads for one batch element, including the
row-sharded output projection; the host sums the two half-partials per batch
and adds the folded bias (bv @ Wo + bo).

Per-core dataflow (fp16 matmuls, fp32 PSUM accumulation, fp8 AV):
  xT [D,T] -> QT/KT [512,T] fp16 (scale 1/8 folded into Wq), V fp8e4 [T,512]
  per head pair (A,B), per 512-wide q chunk, per k token-tile t:
    scoresT[:,A/B] = KT-slice^T x QT-slice   (K=64, A/B concurrent on
    disjoint PE row halves)
    E = exp(scoresT) on ScalarE, PSUM f32 -> SBUF fp8e4, written into the
    t%2 slot of a per-token-pair E tile
  per token-tile pair tp: DoubleRow fp8 matmuls contract both tiles at once:
    num_A/B += V_aug[tp]^T @ E[tp]  (V carries a ones column -> rowsum lands
    in num row 64 (A) / 32 (B)); 2x fewer AV matmul cycles than fp16.
  softmax denominator: copy nums PSUM->SBUF f32 (releases banks), reciprocal
  of the rowsum rows into a [2,QC] fp16 tile, broadcast to 128 partitions
  with a single K=2 fp16 matmul (mask lhsT), multiply -> AOT [512,T] fp16;
  out = AOT^T-slices @ Wo -> PSUM f32 -> out DRAM f32.

Emission is software-pipelined as in the bf16 baseline: projections and the
output projection of finished q-chunks are sprinkled into the per-k-tile
slots of the attention loop; the normalize chain of chunk i is emitted at the
top of chunk i+1.
"""

import numpy as np
import ml_dtypes

P = 128
T = 2048
D = 1024
DH = 512          # per-core head dims (8 heads x 64)
NK = D // P       # 8 contraction tiles for projections
NT = T // P       # 16 token tiles
NTP = NT // 2     # 8 token-tile pairs (fp8 DoubleRow AV)
QC = 512          # q-chunk width
NQC = T // QC     # 4
NPAIR = 4         # head pairs per core
F16 = np.float16
F8 = ml_dtypes.float8_e4m3

_CACHED_NC = None


def build_nc():
    global _CACHED_NC
    if _CACHED_NC is not None:
        return _CACHED_NC
    from contextlib import ExitStack
    import concourse.mybir as mybir
    import concourse.tile as tile
    from concourse import bacc
    from concourse.bass import ds

    f32 = mybir.dt.float32
    fp16 = mybir.dt.float16
    fp8 = mybir.dt.float8e4
    EXP = mybir.ActivationFunctionType.Exp
    DR = mybir.MatmulPerfMode.DoubleRow

    nc = bacc.Bacc("TRN2", target_bir_lowering=False, debug=False, num_devices=8)
    xt_d = nc.dram_tensor("xt", [D, T], fp16, kind="ExternalInput")
    wq_d = nc.dram_tensor("wq", [D, DH], fp16, kind="ExternalInput")
    wk_d = nc.dram_tensor("wk", [D, DH], fp16, kind="ExternalInput")
    wv_d = nc.dram_tensor("wv", [D, DH], fp16, kind="ExternalInput")
    wo_d = nc.dram_tensor("wo", [DH, D], fp16, kind="ExternalInput")
    bq_d = nc.dram_tensor("bq2", [P, NPAIR], f32, kind="ExternalInput")
    bk_d = nc.dram_tensor("bk2", [P, NPAIR], f32, kind="ExternalInput")
    out_d = nc.dram_tensor("out", [T, D], f32, kind="ExternalOutput")

    with tile.TileContext(nc) as tc, ExitStack() as ctx:
        cpool = ctx.enter_context(tc.tile_pool(name="const", bufs=1))
        pp = ctx.enter_context(tc.tile_pool(name="proj", bufs=2, space="PSUM"))
        ps_s = ctx.enter_context(tc.tile_pool(name="scores", bufs=2, space="PSUM"))
        ps_n = ctx.enter_context(tc.tile_pool(name="num", bufs=1, space="PSUM"))
        epool = ctx.enter_context(tc.tile_pool(name="esb", bufs=3))
        npool = ctx.enter_context(tc.tile_pool(name="nsb", bufs=2))
        rpool = ctx.enter_context(tc.tile_pool(name="rsb", bufs=2))
        opool = ctx.enter_context(tc.tile_pool(name="osb", bufs=3))

        xt = [cpool.tile([P, T], fp16, name=f"xt{k}", tag=f"xt{k}") for k in range(NK)]
        wq = cpool.tile([P, NK, DH], fp16, name="wq_s", tag="wq_s")
        wk = cpool.tile([P, NK, DH], fp16, name="wk_s", tag="wk_s")
        wv = cpool.tile([P, NK, DH], fp16, name="wv_s", tag="wv_s")
        wo = cpool.tile([P, NPAIR, D], fp16, name="wo_s", tag="wo_s")
        bq = cpool.tile([P, NPAIR], f32, name="bq_s", tag="bq_s")
        bk = cpool.tile([P, NPAIR], f32, name="bk_s", tag="bk_s")
        # rank-1 lhsT for the denominator broadcast matmuls
        ones1 = cpool.tile([1, 64], fp16, name="ones1", tag="ones1")
        qt = [cpool.tile([P, T], fp16, name=f"qt{r}", tag=f"qt{r}") for r in range(NPAIR)]
        kt = [cpool.tile([P, T], fp16, name=f"kt{r}", tag=f"kt{r}") for r in range(NPAIR)]
        # V layout (fp16), one [.., 128] lhsT block per (token-tile t,
        # head pair r) and half:
        #   A block: [v_A(64), ones, zeros(63)] -> num rows 0:63 = A,
        #            row 64 = rowsum_A
        #   B block: [zeros(32), ones, zeros(31), v_B(64)] -> num rows
        #            64:127 = B, row 32 = rowsum_B
        vsa = cpool.tile([P, NTP, NPAIR, 2, P], fp16, name="va_s", tag="va_s")
        vsbb = cpool.tile([P, NTP, NPAIR, 2, P], fp16, name="vb_s", tag="vb_s")
        aot = [cpool.tile([P, T], fp16, name=f"aot{r}", tag=f"aot{r}") for r in range(NPAIR)]

        # loads: xt early (first projection chains need it), wv/wo last
        nc.sync.dma_start(wq[:], wq_d[:].rearrange("(k p) n -> p k n", p=P))
        nc.sync.dma_start(bq[:], bq_d[:])
        for k in range(NK):
            nc.sync.dma_start(xt[k][:], xt_d[ds(k * P, P), :])
        nc.sync.dma_start(wk[:], wk_d[:].rearrange("(k p) n -> p k n", p=P))
        nc.sync.dma_start(bk[:], bk_d[:])
        nc.sync.dma_start(wv[:], wv_d[:].rearrange("(k p) n -> p k n", p=P))
        nc.sync.dma_start(wo[:], wo_d[:].rearrange("(r p) n -> p r n", p=P))
        nc.vector.memset(ones1[:], 1.0)
        nc.gpsimd.memset(vsa[:, :, :, :, 64:65], 1.0)
        nc.gpsimd.memset(vsa[:, :, :, :, 65:P], 0.0)
        nc.gpsimd.memset(vsbb[:, :, :, :, 0:32], 0.0)
        nc.gpsimd.memset(vsbb[:, :, :, :, 32:33], 1.0)
        nc.gpsimd.memset(vsbb[:, :, :, :, 33:64], 0.0)

        # projection chains are emitted in two halves so a sprinkled chain
        # never inserts more than ~1us of PE work between attention slots
        def proj_qk_a(dst, w, b, r, qc):
            ps = pp.tile([P, QC], f32, name="ps_p", tag="ps_p")
            for k in range(NK // 2):
                nc.tensor.matmul(ps[:], w[:, k, ds(r * P, P)], xt[k][:, ds(qc * QC, QC)],
                                 start=(k == 0), stop=False)
            return ps

        def proj_qk_b(ps, dst, w, b, r, qc):
            for k in range(NK // 2, NK):
                nc.tensor.matmul(ps[:], w[:, k, ds(r * P, P)], xt[k][:, ds(qc * QC, QC)],
                                 start=False, stop=(k == NK - 1))
            nc.vector.tensor_scalar_add(dst[:, ds(qc * QC, QC)], ps[:], b[:, r:r + 1])

        def proj_qk(dst, w, b, r, qc):
            proj_qk_b(proj_qk_a(dst, w, b, r, qc), dst, w, b, r, qc)

        def proj_v_a(t):
            ps = pp.tile([P, DH], f32, name="ps_p", tag="ps_p")
            for k in range(NK // 2):
                nc.tensor.matmul(ps[:], xt[k][:, ds(t * P, P)], wv[:, k, :],
                                 start=(k == 0), stop=False)
            return ps

        def proj_v_b(ps, t):
            for k in range(NK // 2, NK):
                nc.tensor.matmul(ps[:], xt[k][:, ds(t * P, P)], wv[:, k, :],
                                 start=False, stop=(k == NK - 1))
            psv = ps.rearrange("p (r hd) -> p r hd", r=NPAIR)
            nc.vector.tensor_copy(vsa[:, t // 2, :, t % 2, 0:64], psv[:, :, 0:64])
            nc.vector.tensor_copy(vsbb[:, t // 2, :, t % 2, 64:P], psv[:, :, 64:128])

        def proj_v(t):
            proj_v_b(proj_v_a(t), t)

        def proj_out(j, n):
            ps = pp.tile([P, QC], f32, name="ps_p", tag="ps_p")
            for r in range(NPAIR):
                nc.tensor.matmul(ps[:], aot[r][:, ds(j * P, P)], wo[:, r, ds(n * QC, QC)],
                                 start=(r == 0), stop=(r == NPAIR - 1))
            o = opool.tile([P, QC], f32, name="ost", tag="ost")
            nc.vector.tensor_copy(o[:], ps[:])
            nc.sync.dma_start(out_d[ds(j * P, P), ds(n * QC, QC)], o[:])

        def emit_norm_copies(st):
            nA, nB, r, qc = st
            # PSUM -> SBUF copies release the num banks promptly; the rest of
            # the chain runs off the critical path.
            # on the Scalar engine: it idles at window boundaries while the
            # DVE queue is backed up with drained projection copies, and the
            # next window's first AV is blocked on these bank releases
            cA = npool.tile([65, QC], f32, name="cA", tag="cA")
            cB = npool.tile([P, QC], f32, name="cB", tag="cB")
            nc.vector.tensor_copy(cA[:], nA[0:65, :])
            nc.vector.tensor_copy(cB[:], nB[:])
            return (cA, cB, r, qc)

        def emit_norm_rest(st):
            cA, cB, r, qc = st
            r2a = rpool.tile([1, QC], f32, name="r2a", tag="r2a")
            r2b = rpool.tile([1, QC], f32, name="r2b", tag="r2b")
            nc.vector.tensor_copy(r2a[:], cA[64:65, :])
            nc.vector.tensor_copy(r2b[:], cB[32:33, :])
            r3af = rpool.tile([1, QC], f32, name="r3af", tag="r3af")
            r3bf = rpool.tile([1, QC], f32, name="r3bf", tag="r3bf")
            nc.vector.reciprocal_approx_fast(r3af[:], r2a[:])
            nc.vector.reciprocal_approx_fast(r3bf[:], r2b[:])
            r3a = rpool.tile([1, QC], fp16, name="r3a", tag="r3a")
            r3b = rpool.tile([1, QC], fp16, name="r3b", tag="r3b")
            nc.vector.tensor_copy(r3a[:], r3af[:])
            nc.vector.tensor_copy(r3b[:], r3bf[:])
            rbc = pp.tile([P, QC], f32, name="rbc", tag="ps_p")
            nc.tensor.matmul(rbc[0:64, :], ones1[:], r3a[:], start=True, stop=True)
            nc.tensor.matmul(rbc[64:P, :], ones1[:], r3b[:], start=True, stop=True)
            nc.vector.tensor_mul(aot[r][0:64, ds(qc * QC, QC)], cA[0:64, :], rbc[0:64, :])
            nc.vector.tensor_mul(aot[r][64:P, ds(qc * QC, QC)], cB[64:P, :], rbc[64:P, :])

        def emit_normalize(st):
            emit_norm_rest(emit_norm_copies(st))

        def qk_units(dst, w, b, r, qc):
            cell = []
            return [lambda: cell.append(proj_qk_a(dst, w, b, r, qc)),
                    lambda: proj_qk_b(cell[0], dst, w, b, r, qc)]

        def v_units(t):
            cell = []
            return [lambda t=t: cell.append(proj_v_a(t)),
                    lambda t=t: proj_v_b(cell[0], t)]

        # startup: KT spans all k tokens, so the first KT-pair0 chain (plus
        # the first QT chunk and the first two V token-tiles) precede
        # attention; everything else is sprinkled into attention slots.
        proj_qk(qt[0], wq, bq, 0, 0)
        proj_qk(kt[0], wk, bk, 0, 0)
        proj_v(0)
        proj_v(1)

        pending_norm = None
        for r in range(NPAIR):
            for qc in range(NQC):
                # unit queue for this (r, qc) window: each unit is ~1us of
                # PE work, drained a few per slot
                units = []
                if r == 0 and qc == 0:
                    units += qk_units(kt[0], wk, bk, 0, 1)
                    units += v_units(2) + v_units(3)
                    units += qk_units(kt[0], wk, bk, 0, 2)
                    units += v_units(4) + v_units(5)
                    units += qk_units(kt[0], wk, bk, 0, 3)
                    for t in range(6, 9):
                        units += v_units(t)
                    units += qk_units(qt[0], wq, bq, 0, 1)
                    for t in range(9, 12):
                        units += v_units(t)
                    units += qk_units(qt[0], wq, bq, 0, 2)
                    for t in range(12, NT):
                        units += v_units(t)
                    units += qk_units(qt[0], wq, bq, 0, 3)
                elif r == 0:
                    # pair-1 projections squeezed into the 3 remaining
                    # pair-0 windows (KT chains first)
                    chains = [("k", 0), ("k", 1), ("k", 2), ("k", 3),
                              ("q", 0), ("q", 1), ("q", 2), ("q", 3)]
                    for kind, c in chains[(qc - 1) * 3:qc * 3]:
                        dst, w_, b_ = ((kt[1], wk, bk) if kind == "k"
                                       else (qt[1], wq, bq))
                        units += qk_units(dst, w_, b_, 1, c)
                elif r < NPAIR - 1:
                    # next pair's projections: KT chains first (needed from
                    # t=0 of its qc0), QT chains later
                    kind = ("k", kt) if qc < 2 else ("q", qt)
                    w_, b_ = (wk, bk) if qc < 2 else (wq, bq)
                    c0 = 2 * qc if qc < 2 else 2 * (qc - 2)
                    units += qk_units(kind[1][r + 1], w_, b_, r + 1, c0)
                    units += qk_units(kind[1][r + 1], w_, b_, r + 1, c0 + 1)
                if r == NPAIR - 1 and qc > 0:
                    for i in range(8):
                        j = (qc - 1) * NPAIR + i // 2
                        units.append(lambda j=j, n=i % 2: proj_out(j, n))

                nA = nB = None
                es = {}
                ui = 0
                av_done = 0
                for t in range(NT):
                    sc = ps_s.tile([P, 2, QC], f32, name="sc", tag="sc")
                    nc.tensor.matmul(sc[:, 0, :], kt[r][0:64, ds(t * P, P)],
                                     qt[r][0:64, ds(qc * QC, QC)], start=True, stop=True)
                    nc.tensor.matmul(sc[:, 1, :], kt[r][64:P, ds(t * P, P)],
                                     qt[r][64:P, ds(qc * QC, QC)], start=True, stop=True)
                    if t % 2 == 0:
                        es[t // 2] = epool.tile([P, 2, 2, QC], fp16, name="eT", tag="eT")
                    # E slot layout: [partitions, head, t-in-pair, QC] so each
                    # head's two t-slots are a contiguous DoubleRow ifmap
                    nc.scalar.activation(es[t // 2][:, :, t % 2, :], sc[:], EXP)
                    if t == 0 and pending_norm is not None:
                        emit_normalize(pending_norm)
                        pending_norm = None
                    if t >= 1:
                        if t == 1:
                            nA = ps_n.tile([P, QC], f32, name="nA", tag="nA")
                            nB = ps_n.tile([P, QC], f32, name="nB", tag="nB")
                        tv = t - 1
                        nc.tensor.matmul(nA[:], vsa[:, tv // 2, r, tv % 2, :],
                                         es[tv // 2][:, 0, tv % 2, :],
                                         start=(tv == 0), stop=False)
                        nc.tensor.matmul(nB[:], vsbb[:, tv // 2, r, tv % 2, :],
                                         es[tv // 2][:, 1, tv % 2, :],
                                         start=(tv == 0), stop=False)
                        if tv % 2 == 1:
                            del es[tv // 2]
                    # drain the unit queue evenly, finishing by slot 13 so
                    # the DVE queue is clear for the boundary num drain
                    want = -(-(len(units) - ui) // max(1, NT - 2 - t))
                    for _ in range(want):
                        units[ui]()
                        ui += 1
                while ui < len(units):
                    units[ui]()
                    ui += 1
                tv = NT - 1
                nc.tensor.matmul(nA[:], vsa[:, tv // 2, r, tv % 2, :],
                                 es[tv // 2][:, 0, tv % 2, :],
                                 start=False, stop=True)
                nc.tensor.matmul(nB[:], vsbb[:, tv // 2, r, tv % 2, :],
                                 es[tv // 2][:, 1, tv % 2, :],
                                 start=False, stop=True)
                del es[tv // 2]
                pending_norm = (nA, nB, r, qc)

        emit_normalize(pending_norm)
        # tail: output projection of the last q-chunk
        for j in range((NQC - 1) * NPAIR, NT):
            for n in range(D // QC):
                proj_out(j, n)

    nc.compile()

    _CACHED_NC = nc
    return nc


def prepare_in_maps(inputs):
    x = np.asarray(inputs["x"], np.float32)
    Wq = np.asarray(inputs["Wq"], np.float32)
    bq = np.asarray(inputs["bq"], np.float32)
    Wk = np.asarray(inputs["Wk"], np.float32)
    bk = np.asarray(inputs["bk"], np.float32)
    Wv = np.asarray(inputs["Wv"], np.float32)
    Wo = np.asarray(inputs["Wo"], np.float32)
    in_maps = []
    for c in range(8):
        b, half = c // 2, c % 2
        cols = slice(half * DH, (half + 1) * DH)
        in_maps.append({
            "xt": np.ascontiguousarray(x[b].T).astype(F16),
            "wq": np.ascontiguousarray(Wq[:, cols] / 8.0).astype(F16),
            "wk": np.ascontiguousarray(Wk[:, cols]).astype(F16),
            "wv": np.ascontiguousarray(Wv[:, cols]).astype(F16),
            "wo": np.ascontiguousarray(Wo[cols, :]).astype(F16),
            "bq2": np.ascontiguousarray((bq[cols] / 8.0).astype(np.float32).reshape(NPAIR, P).T),
            "bk2": np.ascontiguousarray(bk[cols].astype(np.float32).reshape(NPAIR, P).T),
        })
    return in_maps


def postprocess(results, inputs):
    bv = np.asarray(inputs["bv"], np.float64)
    Wo = np.asarray(inputs["Wo"], np.float64)
    bo = np.asarray(inputs["bo"], np.float64)
    bo_eff = (bv @ Wo + bo).astype(np.float32)
    out = np.empty((4, T, D), np.float32)
    for b in range(4):
        out[b] = (results[2 * b]["out"]
                  + results[2 * b + 1]["out"]
                  + bo_eff[None, :])
    return out


def kernel(**inputs):
    from concourse.bass_utils import run_bass_kernel_spmd
    nc = build_nc()
    in_maps = prepare_in_maps(inputs)
    res = run_bass_kernel_spmd(nc, in_maps, core_ids=list(range(8)))
    return postprocess(res.results, inputs)


# revision 43
# speedup vs baseline: 1.1819x; 1.0250x over previous
"""Multi-head self-attention (B=4, T=2048, D=1024, H=16, Dh=64) on 8 trn2 cores.

Sharding: core c = (batch b = c//2, head-half = c%2). Each core computes the
attention output contribution of 8 heads for one batch element, including the
row-sharded output projection; the host sums the two half-partials per batch
and adds the folded bias (bv @ Wo + bo).

Per-core dataflow (fp16 matmuls, fp32 PSUM accumulation, fp8 AV):
  xT [D,T] -> QT/KT [512,T] fp16 (scale 1/8 folded into Wq), V fp8e4 [T,512]
  per head pair (A,B), per 512-wide q chunk, per k token-tile t:
    scoresT[:,A/B] = KT-slice^T x QT-slice   (K=64, A/B concurrent on
    disjoint PE row halves)
    E = exp(scoresT) on ScalarE, PSUM f32 -> SBUF fp8e4, written into the
    t%2 slot of a per-token-pair E tile
  per token-tile pair tp: DoubleRow fp8 matmuls contract both tiles at once:
    num_A/B += V_aug[tp]^T @ E[tp]  (V carries a ones column -> rowsum lands
    in num row 64 (A) / 32 (B)); 2x fewer AV matmul cycles than fp16.
  softmax denominator: copy nums PSUM->SBUF f32 (releases banks), reciprocal
  of the rowsum rows into a [2,QC] fp16 tile, broadcast to 128 partitions
  with a single K=2 fp16 matmul (mask lhsT), multiply -> AOT [512,T] fp16;
  out = AOT^T-slices @ Wo -> PSUM f32 -> out DRAM f32.

Emission is software-pipelined as in the bf16 baseline: projections and the
output projection of finished q-chunks are sprinkled into the per-k-tile
slots of the attention loop; the normalize chain of chunk i is emitted at the
top of chunk i+1.
"""

import numpy as np
import ml_dtypes

P = 128
T = 2048
D = 1024
DH = 512          # per-core head dims (8 heads x 64)
NK = D // P       # 8 contraction tiles for projections
NT = T // P       # 16 token tiles
NTP = NT // 2     # 8 token-tile pairs (fp8 DoubleRow AV)
QC = 512          # q-chunk width
NQC = T // QC     # 4
NPAIR = 4         # head pairs per core
F16 = np.float16
F8 = ml_dtypes.float8_e4m3

_CACHED_NC = None


def build_nc():
    global _CACHED_NC
    if _CACHED_NC is not None:
        return _CACHED_NC
    from contextlib import ExitStack
    import concourse.mybir as mybir
    import concourse.tile as tile
    from concourse import bacc
    from concourse.bass import ds

    f32 = mybir.dt.float32
    fp16 = mybir.dt.float16
    fp8 = mybir.dt.float8e4
    EXP = mybir.ActivationFunctionType.Exp
    DR = mybir.MatmulPerfMode.DoubleRow

    nc = bacc.Bacc("TRN2", target_bir_lowering=False, debug=False, num_devices=8)
    xt_d = nc.dram_tensor("xt", [D, T], fp16, kind="ExternalInput")
    wq_d = nc.dram_tensor("wq", [D, DH], fp16, kind="ExternalInput")
    wk_d = nc.dram_tensor("wk", [D, DH], fp16, kind="ExternalInput")
    wv_d = nc.dram_tensor("wv", [D, DH], fp16, kind="ExternalInput")
    wo_d = nc.dram_tensor("wo", [DH, D], fp16, kind="ExternalInput")
    bq_d = nc.dram_tensor("bq2", [P, NPAIR], f32, kind="ExternalInput")
    bk_d = nc.dram_tensor("bk2", [P, NPAIR], f32, kind="ExternalInput")
    out_d = nc.dram_tensor("out", [T, D], f32, kind="ExternalOutput")

    with tile.TileContext(nc) as tc, ExitStack() as ctx:
        cpool = ctx.enter_context(tc.tile_pool(name="const", bufs=1))
        pp = ctx.enter_context(tc.tile_pool(name="proj", bufs=2, space="PSUM"))
        ps_s = ctx.enter_context(tc.tile_pool(name="scores", bufs=2, space="PSUM"))
        ps_n = ctx.enter_context(tc.tile_pool(name="num", bufs=1, space="PSUM"))
        epool = ctx.enter_context(tc.tile_pool(name="esb", bufs=3))
        npool = ctx.enter_context(tc.tile_pool(name="nsb", bufs=2))
        rpool = ctx.enter_context(tc.tile_pool(name="rsb", bufs=2))
        opool = ctx.enter_context(tc.tile_pool(name="osb", bufs=3))

        xt = [cpool.tile([P, T], fp16, name=f"xt{k}", tag=f"xt{k}") for k in range(NK)]
        wq = cpool.tile([P, NK, DH], fp16, name="wq_s", tag="wq_s")
        wk = cpool.tile([P, NK, DH], fp16, name="wk_s", tag="wk_s")
        wv = cpool.tile([P, NK, DH], fp16, name="wv_s", tag="wv_s")
        wo = cpool.tile([P, NPAIR, D], fp16, name="wo_s", tag="wo_s")
        bq = cpool.tile([P, NPAIR], f32, name="bq_s", tag="bq_s")
        bk = cpool.tile([P, NPAIR], f32, name="bk_s", tag="bk_s")
        # rank-1 lhsT for the denominator broadcast matmuls
        ones1 = cpool.tile([1, 64], fp16, name="ones1", tag="ones1")
        qt = [cpool.tile([P, T], fp16, name=f"qt{r}", tag=f"qt{r}") for r in range(NPAIR)]
        kt = [cpool.tile([P, T], fp16, name=f"kt{r}", tag=f"kt{r}") for r in range(NPAIR)]
        # V layout (fp16), one [.., 128] lhsT block per (token-tile t,
        # head pair r) and half:
        #   A block: [v_A(64), ones, zeros(63)] -> num rows 0:63 = A,
        #            row 64 = rowsum_A
        #   B block: [zeros(32), ones, zeros(31), v_B(64)] -> num rows
        #            64:127 = B, row 32 = rowsum_B
        vsa = cpool.tile([P, NTP, NPAIR, 2, P], fp16, name="va_s", tag="va_s")
        vsbb = cpool.tile([P, NTP, NPAIR, 2, P], fp16, name="vb_s", tag="vb_s")
        aot = [cpool.tile([P, T], fp16, name=f"aot{r}", tag=f"aot{r}") for r in range(NPAIR)]

        # loads: xt early (first projection chains need it), wv/wo last
        nc.sync.dma_start(wq[:], wq_d[:].rearrange("(k p) n -> p k n", p=P))
        nc.sync.dma_start(bq[:], bq_d[:])
        for k in range(NK):
            nc.sync.dma_start(xt[k][:], xt_d[ds(k * P, P), :])
        nc.sync.dma_start(wk[:], wk_d[:].rearrange("(k p) n -> p k n", p=P))
        nc.sync.dma_start(bk[:], bk_d[:])
        nc.sync.dma_start(wv[:], wv_d[:].rearrange("(k p) n -> p k n", p=P))
        nc.sync.dma_start(wo[:], wo_d[:].rearrange("(r p) n -> p r n", p=P))
        nc.vector.memset(ones1[:], 1.0)
        nc.gpsimd.memset(vsa[:, :, :, :, 64:65], 1.0)
        nc.gpsimd.memset(vsa[:, :, :, :, 65:P], 0.0)
        nc.gpsimd.memset(vsbb[:, :, :, :, 0:32], 0.0)
        nc.gpsimd.memset(vsbb[:, :, :, :, 32:33], 1.0)
        nc.gpsimd.memset(vsbb[:, :, :, :, 33:64], 0.0)

        # projection chains are emitted in two halves so a sprinkled chain
        # never inserts more than ~1us of PE work between attention slots
        def proj_qk_a(dst, w, b, r, qc):
            ps = pp.tile([P, QC], f32, name="ps_p", tag="ps_p")
            for k in range(NK // 2):
                nc.tensor.matmul(ps[:], w[:, k, ds(r * P, P)], xt[k][:, ds(qc * QC, QC)],
                                 start=(k == 0), stop=False)
            return ps

        def proj_qk_b(ps, dst, w, b, r, qc):
            for k in range(NK // 2, NK):
                nc.tensor.matmul(ps[:], w[:, k, ds(r * P, P)], xt[k][:, ds(qc * QC, QC)],
                                 start=False, stop=(k == NK - 1))
            nc.vector.tensor_scalar_add(dst[:, ds(qc * QC, QC)], ps[:], b[:, r:r + 1])

        def proj_qk(dst, w, b, r, qc):
            proj_qk_b(proj_qk_a(dst, w, b, r, qc), dst, w, b, r, qc)

        def proj_v_a(t):
            ps = pp.tile([P, DH], f32, name="ps_p", tag="ps_p")
            for k in range(NK // 2):
                nc.tensor.matmul(ps[:], xt[k][:, ds(t * P, P)], wv[:, k, :],
                                 start=(k == 0), stop=False)
            return ps

        def proj_v_b(ps, t):
            for k in range(NK // 2, NK):
                nc.tensor.matmul(ps[:], xt[k][:, ds(t * P, P)], wv[:, k, :],
                                 start=False, stop=(k == NK - 1))
            psv = ps.rearrange("p (r hd) -> p r hd", r=NPAIR)
            nc.vector.tensor_copy(vsa[:, t // 2, :, t % 2, 0:64], psv[:, :, 0:64])
            nc.vector.tensor_copy(vsbb[:, t // 2, :, t % 2, 64:P], psv[:, :, 64:128])

        def proj_v(t):
            proj_v_b(proj_v_a(t), t)

        def proj_out(j, n):
            ps = pp.tile([P, QC], f32, name="ps_p", tag="ps_p")
            for r in range(NPAIR):
                nc.tensor.matmul(ps[:], aot[r][:, ds(j * P, P)], wo[:, r, ds(n * QC, QC)],
                                 start=(r == 0), stop=(r == NPAIR - 1))
            o = opool.tile([P, QC], f32, name="ost", tag="ost")
            nc.vector.tensor_copy(o[:], ps[:])
            nc.sync.dma_start(out_d[ds(j * P, P), ds(n * QC, QC)], o[:])

        def emit_norm_copies(st):
            nA, nB, r, qc = st
            # PSUM -> SBUF copies release the num banks promptly; the rest of
            # the chain runs off the critical path.
            # on the Scalar engine: it idles at window boundaries while the
            # DVE queue is backed up with drained projection copies, and the
            # next window's first AV is blocked on these bank releases
            cA = npool.tile([65, QC], f32, name="cA", tag="cA")
            cB = npool.tile([P, QC], f32, name="cB", tag="cB")
            nc.vector.tensor_copy(cA[:], nA[0:65, :])
            nc.vector.tensor_copy(cB[:], nB[:])
            return (cA, cB, r, qc)

        def emit_norm_rest(st):
            cA, cB, r, qc = st
            r2a = rpool.tile([1, QC], f32, name="r2a", tag="r2a")
            r2b = rpool.tile([1, QC], f32, name="r2b", tag="r2b")
            nc.vector.tensor_copy(r2a[:], cA[64:65, :])
            nc.vector.tensor_copy(r2b[:], cB[32:33, :])
            r3af = rpool.tile([1, QC], f32, name="r3af", tag="r3af")
            r3bf = rpool.tile([1, QC], f32, name="r3bf", tag="r3bf")
            nc.vector.reciprocal_approx_fast(r3af[:], r2a[:])
            nc.vector.reciprocal_approx_fast(r3bf[:], r2b[:])
            r3a = rpool.tile([1, QC], fp16, name="r3a", tag="r3a")
            r3b = rpool.tile([1, QC], fp16, name="r3b", tag="r3b")
            nc.vector.tensor_copy(r3a[:], r3af[:])
            nc.vector.tensor_copy(r3b[:], r3bf[:])
            rbc = pp.tile([P, QC], f32, name="rbc", tag="ps_p")
            nc.tensor.matmul(rbc[0:64, :], ones1[:], r3a[:], start=True, stop=True)
            nc.tensor.matmul(rbc[64:P, :], ones1[:], r3b[:], start=True, stop=True)
            nc.vector.tensor_mul(aot[r][0:64, ds(qc * QC, QC)], cA[0:64, :], rbc[0:64, :])
            nc.vector.tensor_mul(aot[r][64:P, ds(qc * QC, QC)], cB[64:P, :], rbc[64:P, :])

        def emit_normalize(st):
            emit_norm_rest(emit_norm_copies(st))

        def qk_units(dst, w, b, r, qc):
            cell = []
            return [lambda: cell.append(proj_qk_a(dst, w, b, r, qc)),
                    lambda: proj_qk_b(cell[0], dst, w, b, r, qc)]

        def v_units(t):
            cell = []
            return [lambda t=t: cell.append(proj_v_a(t)),
                    lambda t=t: proj_v_b(cell[0], t)]

        # startup: KT spans all k tokens, so the first KT-pair0 chain (plus
        # the first QT chunk and the first two V token-tiles) precede
        # attention; everything else is sprinkled into attention slots.
        proj_qk(qt[0], wq, bq, 0, 0)
        proj_qk(kt[0], wk, bk, 0, 0)
        proj_v(0)
        proj_v(1)

        pending_norm = None
        for r in range(NPAIR):
            for qc in range(NQC):
                # unit queue for this (r, qc) window: each unit is ~1us of
                # PE work, drained a few per slot
                units = []
                if r == 0 and qc == 0:
                    units += qk_units(kt[0], wk, bk, 0, 1)
                    units += v_units(2) + v_units(3)
                    units += qk_units(kt[0], wk, bk, 0, 2)
                    units += v_units(4) + v_units(5)
                    units += qk_units(kt[0], wk, bk, 0, 3)
                    for t in range(6, 9):
                        units += v_units(t)
                    units += qk_units(qt[0], wq, bq, 0, 1)
                    for t in range(9, 12):
                        units += v_units(t)
                    units += qk_units(qt[0], wq, bq, 0, 2)
                    for t in range(12, NT):
                        units += v_units(t)
                    units += qk_units(qt[0], wq, bq, 0, 3)
                elif r == 0:
                    # pair-1 projections squeezed into the 3 remaining
                    # pair-0 windows (KT chains first)
                    chains = [("k", 0), ("k", 1), ("k", 2), ("k", 3),
                              ("q", 0), ("q", 1), ("q", 2), ("q", 3)]
                    for kind, c in chains[(qc - 1) * 3:qc * 3]:
                        dst, w_, b_ = ((kt[1], wk, bk) if kind == "k"
                                       else (qt[1], wq, bq))
                        units += qk_units(dst, w_, b_, 1, c)
                elif r < NPAIR - 1:
                    # next pair's projections: KT chains first (needed from
                    # t=0 of its qc0), QT chains later
                    kind = ("k", kt) if qc < 2 else ("q", qt)
                    w_, b_ = (wk, bk) if qc < 2 else (wq, bq)
                    c0 = 2 * qc if qc < 2 else 2 * (qc - 2)
                    units += qk_units(kind[1][r + 1], w_, b_, r + 1, c0)
                    units += qk_units(kind[1][r + 1], w_, b_, r + 1, c0 + 1)
                if r == NPAIR - 1 and qc > 0:
                    for i in range(8):
                        j = (qc - 1) * NPAIR + i // 2
                        units.append(lambda j=j, n=i % 2: proj_out(j, n))

                nA = nB = None
                es = {}
                ui = 0
                av_done = 0
                for t in range(NT):
                    sc = ps_s.tile([P, 2, QC], f32, name="sc", tag="sc")
                    nc.tensor.matmul(sc[:, 0, :], kt[r][0:64, ds(t * P, P)],
                                     qt[r][0:64, ds(qc * QC, QC)], start=True, stop=True)
                    nc.tensor.matmul(sc[:, 1, :], kt[r][64:P, ds(t * P, P)],
                                     qt[r][64:P, ds(qc * QC, QC)], start=True, stop=True)
                    if t % 2 == 0:
                        es[t // 2] = epool.tile([P, 2, 2, QC], fp16, name="eT", tag="eT")
                    # E slot layout: [partitions, head, t-in-pair, QC] so each
                    # head's two t-slots are a contiguous DoubleRow ifmap
                    nc.scalar.activation(es[t // 2][:, :, t % 2, :], sc[:], EXP)
                    if t == 0 and pending_norm is not None:
                        emit_normalize(pending_norm)
                        pending_norm = None
                    # AV lags 3 slots so the previous window's num-bank
                    # drain (DVE copies) completes before AV(0) needs the
                    # banks; the last 3 AV pairs run after the loop
                    if t >= 3:
                        if t == 3:
                            nA = ps_n.tile([P, QC], f32, name="nA", tag="nA")
                            nB = ps_n.tile([P, QC], f32, name="nB", tag="nB")
                        tv = t - 3
                        nc.tensor.matmul(nA[:], vsa[:, tv // 2, r, tv % 2, :],
                                         es[tv // 2][:, 0, tv % 2, :],
                                         start=(tv == 0), stop=False)
                        nc.tensor.matmul(nB[:], vsbb[:, tv // 2, r, tv % 2, :],
                                         es[tv // 2][:, 1, tv % 2, :],
                                         start=(tv == 0), stop=False)
                        if tv % 2 == 1:
                            del es[tv // 2]
                    # drain the unit queue evenly, finishing by slot 13 so
                    # the DVE queue is clear for the boundary num drain
                    want = -(-(len(units) - ui) // max(1, NT - 2 - t))
                    for _ in range(want):
                        units[ui]()
                        ui += 1
                while ui < len(units):
                    units[ui]()
                    ui += 1
                for tv in range(NT - 3, NT):
                    nc.tensor.matmul(nA[:], vsa[:, tv // 2, r, tv % 2, :],
                                     es[tv // 2][:, 0, tv % 2, :],
                                     start=False, stop=(tv == NT - 1))
                    nc.tensor.matmul(nB[:], vsbb[:, tv // 2, r, tv % 2, :],
                                     es[tv // 2][:, 1, tv % 2, :],
                                     start=False, stop=(tv == NT - 1))
                    if tv % 2 == 1:
                        del es[tv // 2]
                pending_norm = (nA, nB, r, qc)

        emit_normalize(pending_norm)
        # tail: output projection of the last q-chunk
        for j in range((NQC - 1) * NPAIR, NT):
            for n in range(D // QC):
                proj_out(j, n)

    nc.compile()

    _CACHED_NC = nc
    return nc


def prepare_in_maps(inputs):
    x = np.asarray(inputs["x"], np.float32)
    Wq = np.asarray(inputs["Wq"], np.float32)
    bq = np.asarray(inputs["bq"], np.float32)
    Wk = np.asarray(inputs["Wk"], np.float32)
    bk = np.asarray(inputs["bk"], np.float32)
    Wv = np.asarray(inputs["Wv"], np.float32)
    Wo = np.asarray(inputs["Wo"], np.float32)
    in_maps = []
    for c in range(8):
        b, half = c // 2, c % 2
        cols = slice(half * DH, (half + 1) * DH)
        in_maps.append({
            "xt": np.ascontiguousarray(x[b].T).astype(F16),
            "wq": np.ascontiguousarray(Wq[:, cols] / 8.0).astype(F16),
            "wk": np.ascontiguousarray(Wk[:, cols]).astype(F16),
            "wv": np.ascontiguousarray(Wv[:, cols]).astype(F16),
            "wo": np.ascontiguousarray(Wo[cols, :]).astype(F16),
            "bq2": np.ascontiguousarray((bq[cols] / 8.0).astype(np.float32).reshape(NPAIR, P).T),
            "bk2": np.ascontiguousarray(bk[cols].astype(np.float32).reshape(NPAIR, P).T),
        })
    return in_maps


def postprocess(results, inputs):
    bv = np.asarray(inputs["bv"], np.float64)
    Wo = np.asarray(inputs["Wo"], np.float64)
    bo = np.asarray(inputs["bo"], np.float64)
    bo_eff = (bv @ Wo + bo).astype(np.float32)
    out = np.empty((4, T, D), np.float32)
    for b in range(4):
        out[b] = (results[2 * b]["out"]
                  + results[2 * b + 1]["out"]
                  + bo_eff[None, :])
    return out


def kernel(**inputs):
    from concourse.bass_utils import run_bass_kernel_spmd
    nc = build_nc()
    in_maps = prepare_in_maps(inputs)
    res = run_bass_kernel_spmd(nc, in_maps, core_ids=list(range(8)))
    return postprocess(res.results, inputs)


# revision 49
# speedup vs baseline: 1.2920x; 1.0931x over previous
"""Multi-head self-attention (B=4, T=2048, D=1024, H=16, Dh=64) on 8 trn2 cores.

Sharding: core c = (batch b = c//2, head-half = c%2). Each core computes the
attention output contribution of 8 heads for one batch element, including the
row-sharded output projection; the host sums the two half-partials per batch
and adds the folded bias (bv @ Wo + bo).

Per-core dataflow (fp16 matmuls, fp32 PSUM accumulation, fp8 AV):
  xT [D,T] -> QT/KT [512,T] fp16 (scale 1/8 folded into Wq), V fp8e4 [T,512]
  per head pair (A,B), per 512-wide q chunk, per k token-tile t:
    scoresT[:,A/B] = KT-slice^T x QT-slice   (K=64, A/B concurrent on
    disjoint PE row halves)
    E = exp(scoresT) on ScalarE, PSUM f32 -> SBUF fp8e4, written into the
    t%2 slot of a per-token-pair E tile
  per token-tile pair tp: DoubleRow fp8 matmuls contract both tiles at once:
    num_A/B += V_aug[tp]^T @ E[tp]  (V carries a ones column -> rowsum lands
    in num row 64 (A) / 32 (B)); 2x fewer AV matmul cycles than fp16.
  softmax denominator: copy nums PSUM->SBUF f32 (releases banks), reciprocal
  of the rowsum rows into a [2,QC] fp16 tile, broadcast to 128 partitions
  with a single K=2 fp16 matmul (mask lhsT), multiply -> AOT [512,T] fp16;
  out = AOT^T-slices @ Wo -> PSUM f32 -> out DRAM f32.

Emission is software-pipelined as in the bf16 baseline: projections and the
output projection of finished q-chunks are sprinkled into the per-k-tile
slots of the attention loop; the normalize chain of chunk i is emitted at the
top of chunk i+1.
"""

import numpy as np
import ml_dtypes

P = 128
T = 2048
D = 1024
DH = 512          # per-core head dims (8 heads x 64)
NK = D // P       # 8 contraction tiles for projections
NT = T // P       # 16 token tiles
NTP = NT // 2     # 8 token-tile pairs (fp8 DoubleRow AV)
QC = 512          # q-chunk width
NQC = T // QC     # 4
NPAIR = 4         # head pairs per core
F16 = np.float16
F8 = ml_dtypes.float8_e4m3

_CACHED_NC = None


def build_nc():
    global _CACHED_NC
    if _CACHED_NC is not None:
        return _CACHED_NC
    from contextlib import ExitStack
    import concourse.mybir as mybir
    import concourse.tile as tile
    from concourse import bacc
    from concourse.bass import ds

    f32 = mybir.dt.float32
    fp16 = mybir.dt.float16
    fp8 = mybir.dt.float8e4
    EXP = mybir.ActivationFunctionType.Exp
    DR = mybir.MatmulPerfMode.DoubleRow

    nc = bacc.Bacc("TRN2", target_bir_lowering=False, debug=False, num_devices=8)
    xt_d = nc.dram_tensor("xt", [D, T], fp16, kind="ExternalInput")
    wq_d = nc.dram_tensor("wq", [D, DH], fp16, kind="ExternalInput")
    wk_d = nc.dram_tensor("wk", [D, DH], fp16, kind="ExternalInput")
    wv_d = nc.dram_tensor("wv", [D, DH], fp16, kind="ExternalInput")
    wo_d = nc.dram_tensor("wo", [DH, D], fp16, kind="ExternalInput")
    bq_d = nc.dram_tensor("bq2", [P, NPAIR], f32, kind="ExternalInput")
    bk_d = nc.dram_tensor("bk2", [P, NPAIR], f32, kind="ExternalInput")
    out_d = nc.dram_tensor("out", [T, D], f32, kind="ExternalOutput")

    with tile.TileContext(nc) as tc, ExitStack() as ctx:
        cpool = ctx.enter_context(tc.tile_pool(name="const", bufs=1))
        pp = ctx.enter_context(tc.tile_pool(name="proj", bufs=2, space="PSUM"))
        ps_s = ctx.enter_context(tc.tile_pool(name="scores", bufs=2, space="PSUM"))
        ps_n = ctx.enter_context(tc.tile_pool(name="num", bufs=1, space="PSUM"))
        epool = ctx.enter_context(tc.tile_pool(name="esb", bufs=3))
        npool = ctx.enter_context(tc.tile_pool(name="nsb", bufs=2))
        rpool = ctx.enter_context(tc.tile_pool(name="rsb", bufs=2))
        opool = ctx.enter_context(tc.tile_pool(name="osb", bufs=3))

        xt = [cpool.tile([P, T], fp16, name=f"xt{k}", tag=f"xt{k}") for k in range(NK)]
        wq = cpool.tile([P, NK, DH], fp16, name="wq_s", tag="wq_s")
        wk = cpool.tile([P, NK, DH], fp16, name="wk_s", tag="wk_s")
        wv = cpool.tile([P, NK, DH], fp16, name="wv_s", tag="wv_s")
        wo = cpool.tile([P, NPAIR, D], fp16, name="wo_s", tag="wo_s")
        bq = cpool.tile([P, NPAIR], f32, name="bq_s", tag="bq_s")
        bk = cpool.tile([P, NPAIR], f32, name="bk_s", tag="bk_s")
        # rank-1 lhsT for the denominator broadcast matmuls
        ones1 = cpool.tile([1, 64], fp16, name="ones1", tag="ones1")
        qt = [cpool.tile([P, T], fp16, name=f"qt{r}", tag=f"qt{r}") for r in range(NPAIR)]
        kt = [cpool.tile([P, T], fp16, name=f"kt{r}", tag=f"kt{r}") for r in range(NPAIR)]
        # V layout (fp16), one [.., 128] lhsT block per (token-tile t,
        # head pair r) and half:
        #   A block: [v_A(64), ones, zeros(63)] -> num rows 0:63 = A,
        #            row 64 = rowsum_A
        #   B block: [zeros(32), ones, zeros(31), v_B(64)] -> num rows
        #            64:127 = B, row 32 = rowsum_B
        vsa = cpool.tile([P, NTP, NPAIR, 2, P], fp16, name="va_s", tag="va_s")
        vsbb = cpool.tile([P, NTP, NPAIR, 2, P], fp16, name="vb_s", tag="vb_s")
        aot = [cpool.tile([P, T], fp16, name=f"aot{r}", tag=f"aot{r}") for r in range(NPAIR)]

        # loads: xt early (first projection chains need it), wv/wo last
        nc.sync.dma_start(wq[:], wq_d[:].rearrange("(k p) n -> p k n", p=P))
        nc.sync.dma_start(bq[:], bq_d[:])
        for k in range(NK):
            nc.sync.dma_start(xt[k][:], xt_d[ds(k * P, P), :])
        nc.sync.dma_start(wk[:], wk_d[:].rearrange("(k p) n -> p k n", p=P))
        nc.sync.dma_start(bk[:], bk_d[:])
        nc.sync.dma_start(wv[:], wv_d[:].rearrange("(k p) n -> p k n", p=P))
        nc.sync.dma_start(wo[:], wo_d[:].rearrange("(r p) n -> p r n", p=P))
        nc.vector.memset(ones1[:], 1.0)
        nc.gpsimd.memset(vsa[:, :, :, :, 64:65], 1.0)
        nc.gpsimd.memset(vsa[:, :, :, :, 65:P], 0.0)
        nc.gpsimd.memset(vsbb[:, :, :, :, 0:32], 0.0)
        nc.gpsimd.memset(vsbb[:, :, :, :, 32:33], 1.0)
        nc.gpsimd.memset(vsbb[:, :, :, :, 33:64], 0.0)

        # projection chains are emitted in two halves so a sprinkled chain
        # never inserts more than ~1us of PE work between attention slots
        def proj_qk_a(dst, w, b, r, qc):
            ps = pp.tile([P, QC], f32, name="ps_p", tag="ps_p")
            for k in range(NK // 2):
                nc.tensor.matmul(ps[:], w[:, k, ds(r * P, P)], xt[k][:, ds(qc * QC, QC)],
                                 start=(k == 0), stop=False)
            return ps

        def proj_qk_b(ps, dst, w, b, r, qc):
            for k in range(NK // 2, NK):
                nc.tensor.matmul(ps[:], w[:, k, ds(r * P, P)], xt[k][:, ds(qc * QC, QC)],
                                 start=False, stop=(k == NK - 1))
            nc.vector.tensor_scalar_add(dst[:, ds(qc * QC, QC)], ps[:], b[:, r:r + 1])

        def proj_qk(dst, w, b, r, qc):
            proj_qk_b(proj_qk_a(dst, w, b, r, qc), dst, w, b, r, qc)

        def proj_v_a(t):
            ps = pp.tile([P, DH], f32, name="ps_p", tag="ps_p")
            for k in range(NK // 2):
                nc.tensor.matmul(ps[:], xt[k][:, ds(t * P, P)], wv[:, k, :],
                                 start=(k == 0), stop=False)
            return ps

        def proj_v_b(ps, t):
            for k in range(NK // 2, NK):
                nc.tensor.matmul(ps[:], xt[k][:, ds(t * P, P)], wv[:, k, :],
                                 start=False, stop=(k == NK - 1))
            psv = ps.rearrange("p (r hd) -> p r hd", r=NPAIR)
            nc.vector.tensor_copy(vsa[:, t // 2, :, t % 2, 0:64], psv[:, :, 0:64])
            nc.vector.tensor_copy(vsbb[:, t // 2, :, t % 2, 64:P], psv[:, :, 64:128])

        def proj_v(t):
            proj_v_b(proj_v_a(t), t)

        def proj_out(j, n):
            ps = pp.tile([P, QC], f32, name="ps_p", tag="ps_p")
            for r in range(NPAIR):
                nc.tensor.matmul(ps[:], aot[r][:, ds(j * P, P)], wo[:, r, ds(n * QC, QC)],
                                 start=(r == 0), stop=(r == NPAIR - 1))
            o = opool.tile([P, QC], f32, name="ost", tag="ost")
            nc.vector.tensor_copy(o[:], ps[:])
            nc.sync.dma_start(out_d[ds(j * P, P), ds(n * QC, QC)], o[:])

        def emit_norm_copies(st):
            nA, nB, r, qc = st
            # PSUM -> SBUF copies release the num banks promptly; the rest of
            # the chain runs off the critical path.
            # on the Scalar engine: it idles at window boundaries while the
            # DVE queue is backed up with drained projection copies, and the
            # next window's first AV is blocked on these bank releases
            cA = npool.tile([65, QC], f32, name="cA", tag="cA")
            cB = npool.tile([P, QC], f32, name="cB", tag="cB")
            nc.vector.tensor_copy(cA[:], nA[0:65, :])
            nc.vector.tensor_copy(cB[:], nB[:])
            return (cA, cB, r, qc)

        def emit_norm_rest(st):
            cA, cB, r, qc = st
            r2a = rpool.tile([1, QC], f32, name="r2a", tag="r2a")
            r2b = rpool.tile([1, QC], f32, name="r2b", tag="r2b")
            nc.vector.tensor_copy(r2a[:], cA[64:65, :])
            nc.vector.tensor_copy(r2b[:], cB[32:33, :])
            r3af = rpool.tile([1, QC], f32, name="r3af", tag="r3af")
            r3bf = rpool.tile([1, QC], f32, name="r3bf", tag="r3bf")
            nc.vector.reciprocal_approx_fast(r3af[:], r2a[:])
            nc.vector.reciprocal_approx_fast(r3bf[:], r2b[:])
            r3a = rpool.tile([1, QC], fp16, name="r3a", tag="r3a")
            r3b = rpool.tile([1, QC], fp16, name="r3b", tag="r3b")
            nc.vector.tensor_copy(r3a[:], r3af[:])
            nc.vector.tensor_copy(r3b[:], r3bf[:])
            rbc = pp.tile([P, QC], f32, name="rbc", tag="ps_p")
            nc.tensor.matmul(rbc[0:64, :], ones1[:], r3a[:], start=True, stop=True)
            nc.tensor.matmul(rbc[64:P, :], ones1[:], r3b[:], start=True, stop=True)
            nc.vector.tensor_mul(aot[r][0:64, ds(qc * QC, QC)], cA[0:64, :], rbc[0:64, :])
            nc.vector.tensor_mul(aot[r][64:P, ds(qc * QC, QC)], cB[64:P, :], rbc[64:P, :])

        def emit_normalize(st):
            emit_norm_rest(emit_norm_copies(st))

        def qk_units(dst, w, b, r, qc):
            cell = []
            return [lambda: cell.append(proj_qk_a(dst, w, b, r, qc)),
                    lambda: proj_qk_b(cell[0], dst, w, b, r, qc)]

        def v_units(t):
            cell = []
            return [lambda t=t: cell.append(proj_v_a(t)),
                    lambda t=t: proj_v_b(cell[0], t)]

        # startup: KT spans all k tokens, so the first KT-pair0 chain (plus
        # the first QT chunk and the first two V token-tiles) precede
        # attention; everything else is sprinkled into attention slots.
        proj_qk(qt[0], wq, bq, 0, 0)
        proj_qk(kt[0], wk, bk, 0, 0)
        proj_v(0)
        proj_v(1)

        pending_av = []
        pending_norm = None
        pending_copies = None
        for r in range(NPAIR):
            for qc in range(NQC):
                # unit queue for this (r, qc) window: each unit is ~1us of
                # PE work, drained a few per slot
                units = []
                if r == 0 and qc == 0:
                    units += qk_units(kt[0], wk, bk, 0, 1)
                    units += v_units(2) + v_units(3)
                    units += qk_units(kt[0], wk, bk, 0, 2)
                    units += v_units(4) + v_units(5)
                    units += qk_units(kt[0], wk, bk, 0, 3)
                    for t in range(6, 9):
                        units += v_units(t)
                    units += qk_units(qt[0], wq, bq, 0, 1)
                    for t in range(9, 12):
                        units += v_units(t)
                    units += qk_units(qt[0], wq, bq, 0, 2)
                    for t in range(12, NT):
                        units += v_units(t)
                    units += qk_units(qt[0], wq, bq, 0, 3)
                elif r == 0:
                    # pair-1 projections squeezed into the 3 remaining
                    # pair-0 windows (KT chains first)
                    chains = [("k", 0), ("k", 1), ("k", 2), ("k", 3),
                              ("q", 0), ("q", 1), ("q", 2), ("q", 3)]
                    for kind, c in chains[(qc - 1) * 3:qc * 3]:
                        dst, w_, b_ = ((kt[1], wk, bk) if kind == "k"
                                       else (qt[1], wq, bq))
                        units += qk_units(dst, w_, b_, 1, c)
                elif r < NPAIR - 1:
                    # next pair's projections: KT chains first (needed from
                    # t=0 of its qc0), QT chains later
                    kind = ("k", kt) if qc < 2 else ("q", qt)
                    w_, b_ = (wk, bk) if qc < 2 else (wq, bq)
                    c0 = 2 * qc if qc < 2 else 2 * (qc - 2)
                    units += qk_units(kind[1][r + 1], w_, b_, r + 1, c0)
                    units += qk_units(kind[1][r + 1], w_, b_, r + 1, c0 + 1)
                if r == NPAIR - 1 and qc > 0:
                    for i in range(8):
                        j = (qc - 1) * NPAIR + i // 2
                        units.append(lambda j=j, n=i % 2: proj_out(j, n))

                nA = nB = None
                es = {}
                ui = 0
                av_done = 0
                for t in range(NT):
                    sc = ps_s.tile([P, 2, QC], f32, name="sc", tag="sc")
                    nc.tensor.matmul(sc[:, 0, :], kt[r][0:64, ds(t * P, P)],
                                     qt[r][0:64, ds(qc * QC, QC)], start=True, stop=True)
                    nc.tensor.matmul(sc[:, 1, :], kt[r][64:P, ds(t * P, P)],
                                     qt[r][64:P, ds(qc * QC, QC)], start=True, stop=True)
                    if t % 2 == 0:
                        es[t // 2] = epool.tile([P, 2, 2, QC], fp16, name="eT", tag="eT")
                    # E slot layout: [partitions, head, t-in-pair, QC] so each
                    # head's two t-slots are a contiguous DoubleRow ifmap
                    nc.scalar.activation(es[t // 2][:, :, t % 2, :], sc[:], EXP)
                    # previous window's last 3 AV pairs ride in slots 0-2 so
                    # they never separate exp(15) from the next exp chain;
                    # its normalize follows strictly after (copies at t==3,
                    # the PE broadcast + muls at t==5) to keep the in-order
                    # PE queue free of instructions that wait on the chain
                    if t <= 2 and pending_av:
                        pending_av.pop(0)()
                    if t == 3 and pending_norm is not None:
                        pending_copies = emit_norm_copies(pending_norm)
                        pending_norm = None
                    if t == 5 and pending_copies is not None:
                        emit_norm_rest(pending_copies)
                        pending_copies = None
                    # AV lags 3 slots so the previous window's num-bank
                    # drain (DVE copies) completes before AV(0) needs the
                    # banks; the last 3 AV pairs run after the loop
                    if t >= 3:
                        if t == 3:
                            nA = ps_n.tile([P, QC], f32, name="nA", tag="nA")
                            nB = ps_n.tile([P, QC], f32, name="nB", tag="nB")
                        tv = t - 3
                        nc.tensor.matmul(nA[:], vsa[:, tv // 2, r, tv % 2, :],
                                         es[tv // 2][:, 0, tv % 2, :],
                                         start=(tv == 0), stop=False)
                        nc.tensor.matmul(nB[:], vsbb[:, tv // 2, r, tv % 2, :],
                                         es[tv // 2][:, 1, tv % 2, :],
                                         start=(tv == 0), stop=False)
                        if tv % 2 == 1:
                            del es[tv // 2]
                    # drain the unit queue evenly, finishing by slot 13 so
                    # the DVE queue is clear for the boundary num drain; in
                    # out-proj windows hold off until the deferred normalize
                    # (t==5) has produced this chunk's aot
                    if r < NPAIR - 1 or qc == 0 or t >= 6:
                        want = -(-(len(units) - ui) // max(1, NT - 2 - t))
                        for _ in range(want):
                            units[ui]()
                            ui += 1
                while ui < len(units):
                    units[ui]()
                    ui += 1
                def av_tail(tv, nA=nA, nB=nB, r=r, esl=dict(es)):
                    nc.tensor.matmul(nA[:], vsa[:, tv // 2, r, tv % 2, :],
                                     esl[tv // 2][:, 0, tv % 2, :],
                                     start=False, stop=(tv == NT - 1))
                    nc.tensor.matmul(nB[:], vsbb[:, tv // 2, r, tv % 2, :],
                                     esl[tv // 2][:, 1, tv % 2, :],
                                     start=False, stop=(tv == NT - 1))
                pending_av = [lambda tv=tv: av_tail(tv) for tv in range(NT - 3, NT)]
                es.clear()
                pending_norm = (nA, nB, r, qc)

        for fn in pending_av:
            fn()
        emit_normalize(pending_norm)
        # tail: output projection of the last q-chunk
        for j in range((NQC - 1) * NPAIR, NT):
            for n in range(D // QC):
                proj_out(j, n)

    nc.compile()

    _CACHED_NC = nc
    return nc


def prepare_in_maps(inputs):
    x = np.asarray(inputs["x"], np.float32)
    Wq = np.asarray(inputs["Wq"], np.float32)
    bq = np.asarray(inputs["bq"], np.float32)
    Wk = np.asarray(inputs["Wk"], np.float32)
    bk = np.asarray(inputs["bk"], np.float32)
    Wv = np.asarray(inputs["Wv"], np.float32)
    Wo = np.asarray(inputs["Wo"], np.float32)
    in_maps = []
    for c in range(8):
        b, half = c // 2, c % 2
        cols = slice(half * DH, (half + 1) * DH)
        in_maps.append({
            "xt": np.ascontiguousarray(x[b].T).astype(F16),
            "wq": np.ascontiguousarray(Wq[:, cols] / 8.0).astype(F16),
            "wk": np.ascontiguousarray(Wk[:, cols]).astype(F16),
            "wv": np.ascontiguousarray(Wv[:, cols]).astype(F16),
            "wo": np.ascontiguousarray(Wo[cols, :]).astype(F16),
            "bq2": np.ascontiguousarray((bq[cols] / 8.0).astype(np.float32).reshape(NPAIR, P).T),
            "bk2": np.ascontiguousarray(bk[cols].astype(np.float32).reshape(NPAIR, P).T),
        })
    return in_maps


def postprocess(results, inputs):
    bv = np.asarray(inputs["bv"], np.float64)
    Wo = np.asarray(inputs["Wo"], np.float64)
    bo = np.asarray(inputs["bo"], np.float64)
    bo_eff = (bv @ Wo + bo).astype(np.float32)
    out = np.empty((4, T, D), np.float32)
    for b in range(4):
        out[b] = (results[2 * b]["out"]
                  + results[2 * b + 1]["out"]
                  + bo_eff[None, :])
    return out


def kernel(**inputs):
    from concourse.bass_utils import run_bass_kernel_spmd
    nc = build_nc()
    in_maps = prepare_in_maps(inputs)
    res = run_bass_kernel_spmd(nc, in_maps, core_ids=list(range(8)))
    return postprocess(res.results, inputs)
